# revision 1
# baseline (speedup 1.0000x reference)
"""kNN hypergraph kernel for Trainium2 (8 NeuronCores, Bass/Tile).

Problem: x [16, 256, 768] f32, k=16.
  flat = x.reshape(4096, 768)
  d2[i,j] = |flat_i - flat_j|^2 ; idx = 16 nearest (incl self)
  hypergraph[i, idx[i,:]] = 1 ; out[b,s,t] = sum_b2 hg[b*256+s, b2*256+t]
Output: [16, 256, 256] f32 (per-row histogram of neighbor_index % 256).

Strategy (row-sharded across 8 cores, 512 rows each):
  - Rank rows by s[i,j] = 2*<x_i, x_j> - |x_j|^2  (= sq_i - d2[i,j]; the
    per-row constant sq_i does not change per-row ranking). The 16 NN are
    the 16 LARGEST s per row.
  - Matmul in fp16 hi/lo split (3 cross terms, ~fp32-accurate products at
    full PE speed): s = 2x_hi@x_hi' + 2x_hi@x_lo' + 2x_lo@x_hi' - sq.
    The -sq hi/lo rows ride as two K=1 matmuls (ones stationary).
  - Top-16 per row: per 512-column block, DVE max8 + match_replace + max8
    gives the block top-16 (pipelines with PE); a tiny combine pass over
    the 8x16 union yields sigma = 16th largest of the row.
  - Neighbor mask (s >= sigma) fused with the first histogram fold, then
    binary-tree adds fold the 16 blocks of 256 (sum over batch axis).
"""

import os

import numpy as np

B, S, D = 16, 256, 768
N = B * S            # 4096 points
NCORES = 8
M = N // NCORES      # 512 rows per core
KT = 6               # K tiles of 128 (768 features); -sq rides as K=1 row
KR = D + 1           # 769 rows in the rhs DRAM tensors (row 768 = -sq)
NT = N // 512        # 8 moving tiles of 512 columns
RT = M // 128        # 4 row-tiles of 128 per core
NEG = -3.0e38        # sentinel: far below any real s value (~|s| < 1e5)

_cache = {}


def _build():
    import concourse.mybir as mybir
    import concourse.tile as tile
    from concourse import bacc

    f32 = mybir.dt.float32
    f16 = mybir.dt.float16
    bf16 = mybir.dt.bfloat16

    nc = bacc.Bacc("TRN2", target_bir_lowering=False, debug=False,
                   num_devices=NCORES)

    rh_d = nc.dram_tensor("rhs_hi", [KR, N], f16, kind="ExternalInput")
    rl_d = nc.dram_tensor("rhs_lo", [KR, N], f16, kind="ExternalInput")
    lh_d = nc.dram_tensor("lhs_hi", [D, M], f16, kind="ExternalInput")
    ll_d = nc.dram_tensor("lhs_lo", [D, M], f16, kind="ExternalInput")
    out_d = nc.dram_tensor("out", [M, S], f32, kind="ExternalOutput")

    with tile.TileContext(nc) as tc:
        with (
            tc.tile_pool(name="weights", bufs=1) as wpool,
            tc.tile_pool(name="s", bufs=2) as spool,
            tc.tile_pool(name="mask", bufs=2) as mpool,
            tc.tile_pool(name="m16", bufs=2) as m16pool,
            tc.tile_pool(name="blk", bufs=3) as blkpool,
            tc.tile_pool(name="m8", bufs=4) as m8pool,
            tc.tile_pool(name="outp", bufs=4) as opool,
            tc.tile_pool(name="psum", bufs=8, space="PSUM") as psum,
        ):
            rh_sb, rl_sb, lh_sb, ll_sb = [], [], [], []
            for ki in range(KT):
                t = wpool.tile([128, N], f16, tag=f"rh{ki}", name=f"rh{ki}")
                nc.sync.dma_start(out=t, in_=rh_d[ki * 128:(ki + 1) * 128, :])
                rh_sb.append(t)
                t = wpool.tile([128, N], f16, tag=f"rl{ki}", name=f"rl{ki}")
                nc.sync.dma_start(out=t, in_=rl_d[ki * 128:(ki + 1) * 128, :])
                rl_sb.append(t)
                t = wpool.tile([128, M], f16, tag=f"lh{ki}", name=f"lh{ki}")
                nc.sync.dma_start(out=t, in_=lh_d[ki * 128:(ki + 1) * 128, :])
                lh_sb.append(t)
                t = wpool.tile([128, M], f16, tag=f"ll{ki}", name=f"ll{ki}")
                nc.sync.dma_start(out=t, in_=ll_d[ki * 128:(ki + 1) * 128, :])
                ll_sb.append(t)
            sq_h = wpool.tile([1, N], f16, tag="sq_h", name="sq_h")
            nc.sync.dma_start(out=sq_h, in_=rh_d[D:D + 1, :])
            sq_l = wpool.tile([1, N], f16, tag="sq_l", name="sq_l")
            nc.sync.dma_start(out=sq_l, in_=rl_d[D:D + 1, :])
            ones = wpool.tile([1, 128], f16, tag="ones", name="ones")
            nc.vector.memset(ones, 1.0)

            for rt in range(RT):
                rsl = slice(rt * 128, (rt + 1) * 128)
                s_sb = spool.tile([128, N], f32, tag="s", name="s_sb")
                m16 = m16pool.tile([128, 8 * 16], f32, tag="m16", name="m16")
                ps = [psum.tile([128, 512], f32, tag="ps", name=f"ps{n}")
                      for n in range(NT)]

                def mm(n, ki, pi):
                    lw, rm = [
                        (lh_sb[ki][:, rsl], rh_sb[ki]),
                        (lh_sb[ki][:, rsl], rl_sb[ki]),
                        (ll_sb[ki][:, rsl], rh_sb[ki]),
                    ][pi]
                    nc.tensor.matmul(
                        ps[n][:, :], lw, rm[:, n * 512:(n + 1) * 512],
                        start=(ki == 0 and pi == 0), stop=False)

                def mm_sq(n):
                    # two K=1 matmuls add the -sq row (hi then lo)
                    nsl = slice(n * 512, (n + 1) * 512)
                    nc.tensor.matmul(ps[n][:, :], ones, sq_h[:, nsl],
                                     start=False, stop=False)
                    nc.tensor.matmul(ps[n][:, :], ones, sq_l[:, nsl],
                                     start=False, stop=True)

                def drain_block(n):
                    # PSUM -> SBUF, then per-block top-16 into m16
                    nsl = slice(n * 512, (n + 1) * 512)
                    nc.scalar.copy(out=s_sb[:, nsl], in_=ps[n][:, :])
                    a8 = m16[:, n * 16:n * 16 + 8]
                    b8 = m16[:, n * 16 + 8:n * 16 + 16]
                    scr = blkpool.tile([128, 512], f32, tag="scr", name="scr")
                    nc.vector.max(out=a8, in_=s_sb[:, nsl])
                    nc.vector.match_replace(out=scr, in_to_replace=a8,
                                            in_values=s_sb[:, nsl],
                                            imm_value=NEG)
                    nc.vector.max(out=b8, in_=scr)

                if rt == 0:
                    # first row-tile: K-outer so PE starts as DMA tiles land
                    for ki in range(KT):
                        for pi in range(3):
                            for n in range(NT):
                                mm(n, ki, pi)
                    for n in range(NT):
                        mm_sq(n)
                        drain_block(n)
                else:
                    # weights resident: N-outer so drains pipeline with PE
                    for n in range(NT):
                        for ki in range(KT):
                            for pi in range(3):
                                mm(n, ki, pi)
                        mm_sq(n)
                        drain_block(n)

                # sigma = 16th largest of the union of block top-16s
                c8 = m8pool.tile([128, 8], f32, tag="c8", name="c8")
                m16x = m16pool.tile([128, 8 * 16], f32, tag="m16x", name="m16x")
                d8 = m8pool.tile([128, 8], f32, tag="d8", name="d8")
                nc.vector.max(out=c8, in_=m16)
                nc.vector.match_replace(out=m16x, in_to_replace=c8,
                                        in_values=m16, imm_value=NEG)
                nc.vector.max(out=d8, in_=m16x)
                sigma = d8[:, 7:8]

                # neighbor mask (s >= sigma), fused with first 2048-fold
                H = N // 2
                mask = mpool.tile([128, H], bf16, tag="mask", name="mask")
                nc.vector.tensor_scalar(mask, s_sb[:, :H], sigma, None,
                                        op0=mybir.AluOpType.is_ge)
                nc.vector.scalar_tensor_tensor(
                    out=mask, in0=s_sb[:, H:], scalar=sigma, in1=mask,
                    op0=mybir.AluOpType.is_ge, op1=mybir.AluOpType.add)
                w = H // 2
                while w > S:
                    nc.vector.tensor_add(mask[:, :w], mask[:, :w],
                                         mask[:, w:2 * w])
                    w //= 2
                o = opool.tile([128, S], f32, tag="o", name="o")
                nc.vector.tensor_add(o, mask[:, :S], mask[:, S:2 * S])
                nc.sync.dma_start(out=out_d[rsl, :], in_=o)

    nc.compile()
    return nc


def _prep_inputs(x):
    flat = np.asarray(x, dtype=np.float32).reshape(N, D)
    sq = (flat.astype(np.float64) ** 2).sum(1).astype(np.float32)

    hi = flat.astype(np.float16)
    lo = (flat - hi.astype(np.float32)).astype(np.float16)
    hi2 = (2.0 * flat).astype(np.float16)
    lo2 = (2.0 * flat - hi2.astype(np.float32)).astype(np.float16)
    nsq_h = (-sq).astype(np.float16)
    nsq_l = (-sq - nsq_h.astype(np.float32)).astype(np.float16)

    rhs_hi = np.empty((KR, N), dtype=np.float16)
    rhs_hi[:D] = hi.T
    rhs_hi[D] = nsq_h
    rhs_lo = np.empty((KR, N), dtype=np.float16)
    rhs_lo[:D] = lo.T
    rhs_lo[D] = nsq_l
    lhs_hi = np.ascontiguousarray(hi2.T)   # [768, 4096]
    lhs_lo = np.ascontiguousarray(lo2.T)
    return rhs_hi, rhs_lo, lhs_hi, lhs_lo


def kernel(x, k):
    assert int(k) == 16
    rhs_hi, rhs_lo, lhs_hi, lhs_lo = _prep_inputs(x)

    if "nc" not in _cache:
        _cache["nc"] = _build()
    nc = _cache["nc"]

    in_maps = [
        {"rhs_hi": rhs_hi, "rhs_lo": rhs_lo,
         "lhs_hi": np.ascontiguousarray(lhs_hi[:, c * M:(c + 1) * M]),
         "lhs_lo": np.ascontiguousarray(lhs_lo[:, c * M:(c + 1) * M])}
        for c in range(NCORES)
    ]

    from concourse.bass_utils import run_bass_kernel_spmd
    trace = bool(os.environ.get("KNN_TRACE"))
    if trace:
        try:
            from antenv.axon_hooks import get_axon_ntff_profile_hook  # noqa
        except ImportError:
            trace = False
    res = run_bass_kernel_spmd(nc, in_maps, core_ids=list(range(NCORES)),
                               trace=trace)
    if trace and res.exec_time_ns is not None:
        print(f"HW exec time: {res.exec_time_ns} ns")
        _cache["exec_time_ns"] = res.exec_time_ns

    out = np.concatenate([r["out"] for r in res.results], axis=0)
    return out.reshape(B, S, S)



# revision 4
# speedup vs baseline: 1.3385x; 1.3385x over previous
"""kNN hypergraph kernel for Trainium2 (8 NeuronCores, Bass/Tile).

Problem: x [16, 256, 768] f32, k=16.
  flat = x.reshape(4096, 768)
  d2[i,j] = |flat_i - flat_j|^2 ; idx = 16 nearest (incl self)
  hypergraph[i, idx[i,:]] = 1 ; out[b,s,t] = sum_b2 hg[b*256+s, b2*256+t]
Output: [16, 256, 256] f32.

Strategy (row-sharded across 8 cores, 512 rows each):
  - Rank rows by s[i,j] = 2<x_i,x_j> - |x_j|^2 (per-row constant sq_i
    dropped). The 16 NN are the 16 LARGEST s per row.
  - s = hi2 @ hi'        (fp16 x fp16, full PE rate, 6 MMs/block)
      + e4m3(2x) @ e5m2(lo')   } both cross terms as fp8 DoubleRow
      + e5m2(lo2) @ e4m3(x')   } matmuls: K=256/instr, ~1.44x rate
      - sq  (fp16 hi/lo pair riding as one K=2 matmul)
    where hi2 = fp16(2x), lo2 = 2x - hi2, hi = fp16(x), lo = x - hi.
    Residual error ~1e-3 (vs top-16 decision gaps ~1e-1): exact ranking.
  - Column-chunk-outer loop (8 chunks of 512 cols x 4 row-tiles):
    each moving chunk is one contiguous-per-partition DMA, consumed by
    4 row-tiles -> DMA stays far ahead of PE.
  - Top-16 per row: DVE max8 per 512-block straight from PSUM (top-8 of
    each block; >8 of the true top-16 in one block is a ~1e-3 event and
    within the rel-err budget), then a tiny combine over the 8x8 union
    gives sigma = 16th largest. ACT drains PSUM->SBUF in parallel.
  - Mask (s >= sigma) + log-tree fold over the 16 batches of 256 cols;
    finalize work split across DVE and GpSimd to shorten the tail.
"""

import os

import numpy as np

B, S, D = 16, 256, 768
N = B * S            # 4096 points
NCORES = 8
M = N // NCORES      # 512 rows per core
JT = 6               # K planes of 128 (768 features)
NT = 8               # moving chunks of 512 columns
RT = M // 128        # 4 row-tiles of 128 per core
NEG = -3.0e38        # sentinel: far below any real s value

_cache = {}


def _build():
    import concourse.mybir as mybir
    import concourse.tile as tile
    from concourse import bacc

    f32 = mybir.dt.float32
    f16 = mybir.dt.float16
    bf16 = mybir.dt.bfloat16
    f8e4 = mybir.dt.float8e4
    f8e5 = mybir.dt.float8e5
    DR = mybir.MatmulPerfMode.DoubleRow

    nc = bacc.Bacc("TRN2", target_bir_lowering=False, debug=False,
                   num_devices=NCORES)

    # moving side (identical on all cores), chunk-major so each chunk is
    # one contiguous [128, 3072] DMA
    m16_d = nc.dram_tensor("m16", [NT * 128, JT * 512], f16,
                           kind="ExternalInput")
    mlo_d = nc.dram_tensor("mlo", [NT * 128, JT * 512], f8e5,
                           kind="ExternalInput")
    mhi_d = nc.dram_tensor("mhi", [NT * 128, JT * 512], f8e4,
                           kind="ExternalInput")
    nsq_d = nc.dram_tensor("nsq", [2, N], f16, kind="ExternalInput")
    # stationary side (this core's 512 rows)
    st16_d = nc.dram_tensor("st16", [128, JT * 512], f16,
                            kind="ExternalInput")
    sta_d = nc.dram_tensor("sta", [128, JT * 512], f8e4,
                           kind="ExternalInput")
    stb_d = nc.dram_tensor("stb", [128, JT * 512], f8e5,
                           kind="ExternalInput")
    out_d = nc.dram_tensor("out", [M, S], f32, kind="ExternalOutput")

    with tile.TileContext(nc) as tc:
        with (
            tc.tile_pool(name="weights", bufs=1) as wpool,
            tc.tile_pool(name="s", bufs=1) as spool,
            tc.tile_pool(name="m8", bufs=1) as m8pool,
            tc.tile_pool(name="fin", bufs=2) as fpool,
            tc.tile_pool(name="mask", bufs=2) as mpool,
            tc.tile_pool(name="outp", bufs=2) as opool,
            tc.tile_pool(name="psum", bufs=6, space="PSUM") as psum,
        ):
            # stationaries first (needed by every block)
            st16 = wpool.tile([128, JT, 512], f16, tag="st16", name="st16")
            nc.sync.dma_start(out=st16, in_=st16_d[:, :])
            sta = wpool.tile([128, JT, 512], f8e4, tag="sta", name="sta")
            nc.sync.dma_start(out=sta, in_=sta_d[:, :])
            stb = wpool.tile([128, JT, 512], f8e5, tag="stb", name="stb")
            nc.sync.dma_start(out=stb, in_=stb_d[:, :])
            nsq = wpool.tile([2, N], f16, tag="nsq", name="nsq")
            nc.sync.dma_start(out=nsq, in_=nsq_d[:, :])
            # moving chunks in consumption order
            m16_sb, mlo_sb, mhi_sb = [], [], []
            for n in range(NT):
                t = wpool.tile([128, JT, 512], f16, tag=f"m16_{n}",
                               name=f"m16_{n}")
                nc.sync.dma_start(out=t, in_=m16_d[n * 128:(n + 1) * 128, :])
                m16_sb.append(t)
                t = wpool.tile([128, JT, 512], f8e5, tag=f"mlo_{n}",
                               name=f"mlo_{n}")
                nc.sync.dma_start(out=t, in_=mlo_d[n * 128:(n + 1) * 128, :])
                mlo_sb.append(t)
                t = wpool.tile([128, JT, 512], f8e4, tag=f"mhi_{n}",
                               name=f"mhi_{n}")
                nc.sync.dma_start(out=t, in_=mhi_d[n * 128:(n + 1) * 128, :])
                mhi_sb.append(t)

            ones2 = wpool.tile([2, 128], f16, tag="ones2", name="ones2")
            nc.vector.memset(ones2, 1.0)

            s_sb = [spool.tile([128, N], f32, tag=f"s{rt}", name=f"s{rt}")
                    for rt in range(RT)]
            m8s = [m8pool.tile([128, 64], f32, tag=f"m8_{rt}",
                               name=f"m8_{rt}") for rt in range(RT)]

            def finalize(rt):
                # sigma = 16th largest of the union of the 8 block top-8s
                c8 = fpool.tile([128, 8], f32, tag="c8", name="c8")
                m8x = fpool.tile([128, 64], f32, tag="m8x", name="m8x")
                d8 = fpool.tile([128, 8], f32, tag="d8", name="d8")
                nc.vector.max(out=c8, in_=m8s[rt])
                nc.vector.match_replace(out=m8x, in_to_replace=c8,
                                        in_values=m8s[rt], imm_value=NEG)
                nc.vector.max(out=d8, in_=m8x)
                sigma = d8[:, 7:8]

                # neighbor mask (s >= sigma) fused with first 2048-fold
                eng = nc.vector
                H = N // 2
                mask = mpool.tile([128, H], bf16, tag=f"mask{rt % 2}",
                                  name=f"mask{rt}")
                eng.tensor_scalar(mask, s_sb[rt][:, :H], sigma, None,
                                  op0=mybir.AluOpType.is_ge)
                eng.scalar_tensor_tensor(
                    out=mask, in0=s_sb[rt][:, H:], scalar=sigma, in1=mask,
                    op0=mybir.AluOpType.is_ge, op1=mybir.AluOpType.add)
                w = H // 2
                while w > S:
                    eng.tensor_add(mask[:, :w], mask[:, :w],
                                   mask[:, w:2 * w])
                    w //= 2
                o = opool.tile([128, S], f32, tag=f"o{rt % 2}",
                               name=f"o{rt}")
                eng.tensor_add(o, mask[:, :S], mask[:, S:2 * S])
                nc.sync.dma_start(
                    out=out_d[rt * 128:(rt + 1) * 128, :], in_=o)

            for n in range(NT):
                nsl = slice(n * 512, (n + 1) * 512)
                for rt in range(RT):
                    rsl = slice(rt * 128, (rt + 1) * 128)
                    ps = psum.tile([128, 512], f32, tag="ps", name="ps")
                    for j in range(JT):
                        nc.tensor.matmul(
                            ps, st16[:, j:j + 1, rsl],
                            m16_sb[n][:, j:j + 1, :],
                            start=(j == 0), stop=False)
                    for c in range(JT // 2):
                        nc.tensor.matmul(
                            ps, sta[:, 2 * c:2 * c + 2, rsl],
                            mlo_sb[n][:, 2 * c:2 * c + 2, :],
                            start=False, stop=False, perf_mode=DR)
                    for c in range(JT // 2):
                        nc.tensor.matmul(
                            ps, stb[:, 2 * c:2 * c + 2, rsl],
                            mhi_sb[n][:, 2 * c:2 * c + 2, :],
                            start=False, stop=False, perf_mode=DR)
                    nc.tensor.matmul(ps, ones2, nsq[:, nsl],
                                     start=False, stop=True)
                    # ACT drains PSUM->SBUF while DVE takes the block top-8
                    nc.scalar.copy(out=s_sb[rt][:, nsl], in_=ps)
                    nc.vector.max(out=m8s[rt][:, n * 8:(n + 1) * 8], in_=ps)
                    if n == NT - 1:
                        finalize(rt)

    nc.compile()
    return nc


def _prep_inputs(x):
    import ml_dtypes
    e4 = ml_dtypes.float8_e4m3
    e5 = ml_dtypes.float8_e5m2

    flat = np.asarray(x, dtype=np.float32).reshape(N, D)
    sq = (flat.astype(np.float64) ** 2).sum(1)

    hi = flat.astype(np.float16)                      # rhs fp16
    lo = flat - hi.astype(np.float32)                 # rhs residual
    hi2 = (2.0 * flat).astype(np.float16)             # lhs fp16
    lo2 = 2.0 * flat - hi2.astype(np.float32)         # lhs residual
    nsq_h = (-sq).astype(np.float16)
    nsq_l = (-sq - nsq_h.astype(np.float64)).astype(np.float16)

    def planes(a):
        # [4096, 768] -> [128, 6, 4096]: plane j row p = feature j*128+p
        return np.ascontiguousarray(
            a.T.reshape(JT, 128, N).transpose(1, 0, 2))

    def chunks(a):
        # [128, 6, 4096] -> [1024, 3072]: chunk-major moving layout
        return np.ascontiguousarray(
            a.reshape(128, JT, NT, 512).transpose(2, 0, 1, 3)
            .reshape(NT * 128, JT * 512))

    m16 = chunks(planes(hi))                          # fp16
    mlo = chunks(planes(lo.astype(e5).astype(np.float32))).astype(e5)
    mhi = chunks(planes(flat.astype(e4).astype(np.float32))).astype(e4)
    nsq = np.stack([nsq_h, nsq_l]).astype(np.float16)

    st16_full = planes(hi2)                           # [128, 6, 4096] fp16
    sta_full = planes((2.0 * flat).astype(e4).astype(np.float32))
    stb_full = planes(lo2.astype(e5).astype(np.float32))

    def st_core(a, c, dt):
        return np.ascontiguousarray(
            a[:, :, c * M:(c + 1) * M].reshape(128, JT * 512)).astype(dt)

    return m16, mlo, mhi, nsq, st16_full, sta_full, stb_full, st_core


def kernel(x, k):
    assert int(k) == 16
    import ml_dtypes
    e4 = ml_dtypes.float8_e4m3
    e5 = ml_dtypes.float8_e5m2
    (m16, mlo, mhi, nsq,
     st16_full, sta_full, stb_full, st_core) = _prep_inputs(x)

    if "nc" not in _cache:
        _cache["nc"] = _build()
    nc = _cache["nc"]

    in_maps = [
        {"m16": m16, "mlo": mlo, "mhi": mhi, "nsq": nsq,
         "st16": st_core(st16_full, c, np.float16),
         "sta": st_core(sta_full, c, e4),
         "stb": st_core(stb_full, c, e5)}
        for c in range(NCORES)
    ]

    from concourse.bass_utils import run_bass_kernel_spmd
    trace = bool(os.environ.get("KNN_TRACE"))
    if trace:
        try:
            from antenv.axon_hooks import get_axon_ntff_profile_hook
            if get_axon_ntff_profile_hook() is None:
                trace = False
        except ImportError:
            trace = False
    res = run_bass_kernel_spmd(nc, in_maps, core_ids=list(range(NCORES)),
                               trace=trace)
    if trace and res.exec_time_ns is not None:
        print(f"HW exec time: {res.exec_time_ns} ns")
        _cache["exec_time_ns"] = res.exec_time_ns

    out = np.concatenate([r["out"] for r in res.results], axis=0)
    return out.reshape(B, S, S)


# revision 7
# speedup vs baseline: 1.5076x; 1.1263x over previous
"""kNN hypergraph kernel for Trainium2 (8 NeuronCores, Bass/Tile).

Problem: x [16, 256, 768] f32, k=16.
  flat = x.reshape(4096, 768)
  d2[i,j] = |flat_i - flat_j|^2 ; idx = 16 nearest (incl self)
  hypergraph[i, idx[i,:]] = 1 ; out[b,s,t] = sum_b2 hg[b*256+s, b2*256+t]
Output: [16, 256, 256] f32.

Strategy (row-sharded across 8 cores, 512 rows each):
  - Rank rows by s[i,j] = 2<x_i,x_j> - |x_j|^2 (per-row constant sq_i
    dropped). The 16 NN are the 16 LARGEST s per row.
  - s = hi2 @ hi'        (fp16 x fp16, full PE rate, 6 MMs/block)
      + e4m3(2x) @ e5m2(lo')   } both cross terms as fp8 DoubleRow
      + e5m2(lo2) @ e4m3(x')   } matmuls: K=256/instr, ~1.44x rate
      - sq  (fp16 hi/lo pair riding as one K=2 matmul)
    where hi2 = fp16(2x), lo2 = 2x - hi2, hi = fp16(x), lo = x - hi.
    Residual error ~1e-3 (vs top-16 decision gaps ~1e-1): exact ranking.
  - Column-chunk-outer loop (8 chunks of 512 cols x 4 row-tiles):
    each moving chunk is one contiguous-per-partition DMA, consumed by
    4 row-tiles -> DMA stays far ahead of PE.
  - Top-16 per row: DVE max8 per 512-block straight from PSUM (top-8 of
    each block; >8 of the true top-16 in one block is a ~1e-3 event and
    within the rel-err budget), then a tiny combine over the 8x8 union
    gives sigma = 16th largest. ACT drains PSUM->SBUF in parallel.
  - Mask (s >= sigma) + log-tree fold over the 16 batches of 256 cols;
    finalize work split across DVE and GpSimd to shorten the tail.
"""

import os

import numpy as np

B, S, D = 16, 256, 768
N = B * S            # 4096 points
NCORES = 8
M = N // NCORES      # 512 rows per core
JT = 6               # K planes of 128 (768 features)
NT = 8               # moving chunks of 512 columns
RT = M // 128        # 4 row-tiles of 128 per core
NEG = -3.0e38        # sentinel: far below any real s value

_cache = {}


def _build():
    import concourse.mybir as mybir
    import concourse.tile as tile
    from concourse import bacc

    f32 = mybir.dt.float32
    f16 = mybir.dt.float16
    bf16 = mybir.dt.bfloat16
    f8e4 = mybir.dt.float8e4
    f8e5 = mybir.dt.float8e5
    DR = mybir.MatmulPerfMode.DoubleRow

    nc = bacc.Bacc("TRN2", target_bir_lowering=False, debug=False,
                   num_devices=NCORES)

    # moving side (identical on all cores), chunk-major so each chunk is
    # one contiguous [128, 3072] DMA
    m16_d = nc.dram_tensor("m16", [NT * 128, JT * 512], f16,
                           kind="ExternalInput")
    mlo_d = nc.dram_tensor("mlo", [NT * 128, JT * 512], f8e5,
                           kind="ExternalInput")
    mhi_d = nc.dram_tensor("mhi", [NT * 128, JT * 512], f8e4,
                           kind="ExternalInput")
    nsq_d = nc.dram_tensor("nsq", [2, N], f16, kind="ExternalInput")
    # stationary side (this core's 512 rows)
    st16_d = nc.dram_tensor("st16", [128, JT * 512], f16,
                            kind="ExternalInput")
    sta_d = nc.dram_tensor("sta", [128, JT * 512], f8e4,
                           kind="ExternalInput")
    stb_d = nc.dram_tensor("stb", [128, JT * 512], f8e5,
                           kind="ExternalInput")
    out_d = nc.dram_tensor("out", [M, S], f32, kind="ExternalOutput")

    with tile.TileContext(nc) as tc:
        with (
            tc.tile_pool(name="weights", bufs=1) as wpool,
            tc.tile_pool(name="s", bufs=1) as spool,
            tc.tile_pool(name="m8", bufs=1) as m8pool,
            tc.tile_pool(name="fin", bufs=2) as fpool,
            tc.tile_pool(name="mask", bufs=2) as mpool,
            tc.tile_pool(name="outp", bufs=2) as opool,
            tc.tile_pool(name="psum", bufs=6, space="PSUM") as psum,
        ):
            # critical path first: the fp16 stationary + first moving chunk,
            # both split so round-0 matmuls can start before the full tile
            # lands (plane j is consumed in emission order)
            st16 = wpool.tile([128, JT, 512], f16, tag="st16", name="st16")
            nc.sync.dma_start(out=st16[:, 0:3, :], in_=st16_d[:, :3 * 512])
            m16_sb = [wpool.tile([128, JT, 512], f16, tag=f"m16_{n}",
                                 name=f"m16_{n}") for n in range(NT)]
            for h in range(3):
                nc.sync.dma_start(
                    out=m16_sb[0][:, 2 * h:2 * h + 2, :],
                    in_=m16_d[0:128, 2 * h * 512:(2 * h + 2) * 512])
            nc.sync.dma_start(out=st16[:, 3:6, :], in_=st16_d[:, 3 * 512:])
            sta = wpool.tile([128, JT, 512], f8e4, tag="sta", name="sta")
            nc.sync.dma_start(out=sta, in_=sta_d[:, :])
            stb = wpool.tile([128, JT, 512], f8e5, tag="stb", name="stb")
            nc.sync.dma_start(out=stb, in_=stb_d[:, :])
            mlo_sb = [wpool.tile([128, JT, 512], f8e5, tag=f"mlo_{n}",
                                 name=f"mlo_{n}") for n in range(NT)]
            mhi_sb = [wpool.tile([128, JT, 512], f8e4, tag=f"mhi_{n}",
                                 name=f"mhi_{n}") for n in range(NT)]
            nc.sync.dma_start(out=mlo_sb[0], in_=mlo_d[0:128, :])
            nc.sync.dma_start(out=mhi_sb[0], in_=mhi_d[0:128, :])
            nsq = wpool.tile([2, N], f16, tag="nsq", name="nsq")
            nc.sync.dma_start(out=nsq, in_=nsq_d[:, :])
            for n in range(1, NT):
                nc.sync.dma_start(out=m16_sb[n],
                                  in_=m16_d[n * 128:(n + 1) * 128, :])
                nc.sync.dma_start(out=mlo_sb[n],
                                  in_=mlo_d[n * 128:(n + 1) * 128, :])
                nc.sync.dma_start(out=mhi_sb[n],
                                  in_=mhi_d[n * 128:(n + 1) * 128, :])

            ones2 = wpool.tile([2, 128], f16, tag="ones2", name="ones2")
            nc.vector.memset(ones2, 1.0)

            s_sb = [spool.tile([128, N], f32, tag=f"s{rt}", name=f"s{rt}")
                    for rt in range(RT)]
            m8s = [m8pool.tile([128, 64], f32, tag=f"m8_{rt}",
                               name=f"m8_{rt}") for rt in range(RT)]

            def finalize(rt):
                # sigma = 16th largest of the union of the 8 block top-8s
                c8 = fpool.tile([128, 8], f32, tag="c8", name="c8")
                m8x = fpool.tile([128, 64], f32, tag="m8x", name="m8x")
                d8 = fpool.tile([128, 8], f32, tag="d8", name="d8")
                nc.vector.max(out=c8, in_=m8s[rt])
                nc.vector.match_replace(out=m8x, in_to_replace=c8,
                                        in_values=m8s[rt], imm_value=NEG)
                nc.vector.max(out=d8, in_=m8x)
                sigma = d8[:, 7:8]

                # neighbor mask (s >= sigma): two single-ALU-pass is_ge
                # halves + one bf16 add beat one fused two-op pass
                eng = nc.vector
                H = N // 2
                mask = mpool.tile([128, H], bf16, tag="maskA",
                                  name=f"maskA{rt}")
                maskB = mpool.tile([128, H], bf16, tag="maskB",
                                   name=f"maskB{rt}")
                eng.tensor_scalar(mask, s_sb[rt][:, :H], sigma, None,
                                  op0=mybir.AluOpType.is_ge)
                eng.tensor_scalar(maskB, s_sb[rt][:, H:], sigma, None,
                                  op0=mybir.AluOpType.is_ge)
                eng.tensor_add(mask, mask, maskB)
                w = H // 2
                while w > S:
                    eng.tensor_add(mask[:, :w], mask[:, :w],
                                   mask[:, w:2 * w])
                    w //= 2
                o = opool.tile([128, S], f32, tag=f"o{rt % 2}",
                               name=f"o{rt}")
                eng.tensor_add(o, mask[:, :S], mask[:, S:2 * S])
                nc.sync.dma_start(
                    out=out_d[rt * 128:(rt + 1) * 128, :], in_=o)

            def block(n, rt):
                nsl = slice(n * 512, (n + 1) * 512)
                rsl = slice(rt * 128, (rt + 1) * 128)
                ps = psum.tile([128, 512], f32, tag="ps", name="ps")
                for j in range(JT):
                    nc.tensor.matmul(
                        ps, st16[:, j:j + 1, rsl],
                        m16_sb[n][:, j:j + 1, :],
                        start=(j == 0), stop=False)
                for c in range(JT // 2):
                    nc.tensor.matmul(
                        ps, sta[:, 2 * c:2 * c + 2, rsl],
                        mlo_sb[n][:, 2 * c:2 * c + 2, :],
                        start=False, stop=False, perf_mode=DR)
                for c in range(JT // 2):
                    nc.tensor.matmul(
                        ps, stb[:, 2 * c:2 * c + 2, rsl],
                        mhi_sb[n][:, 2 * c:2 * c + 2, :],
                        start=False, stop=False, perf_mode=DR)
                nc.tensor.matmul(ps, ones2, nsq[:, nsl],
                                 start=False, stop=True)
                # ACT drains PSUM->SBUF while DVE takes the block top-8
                nc.scalar.copy(out=s_sb[rt][:, nsl], in_=ps)
                nc.vector.max(out=m8s[rt][:, n * 8:(n + 1) * 8], in_=ps)

            # phase A: chunk-outer over the first NT/2 chunks (each moving
            # chunk is reused by all 4 row-tiles right after it lands)
            for n in range(NT // 2):
                for rt in range(RT):
                    block(n, rt)
            # phase B: row-outer so each row-tile finishes all its chunks
            # early and its finalize overlaps the next row-tile's matmuls
            for rt in range(RT):
                for n in range(NT // 2, NT):
                    block(n, rt)
                finalize(rt)

    nc.compile()
    return nc


def _prep_inputs(x):
    import ml_dtypes
    e4 = ml_dtypes.float8_e4m3
    e5 = ml_dtypes.float8_e5m2

    flat = np.asarray(x, dtype=np.float32).reshape(N, D)
    sq = (flat.astype(np.float64) ** 2).sum(1)

    hi = flat.astype(np.float16)                      # rhs fp16
    lo = flat - hi.astype(np.float32)                 # rhs residual
    hi2 = (2.0 * flat).astype(np.float16)             # lhs fp16
    lo2 = 2.0 * flat - hi2.astype(np.float32)         # lhs residual
    nsq_h = (-sq).astype(np.float16)
    nsq_l = (-sq - nsq_h.astype(np.float64)).astype(np.float16)

    def planes(a):
        # [4096, 768] -> [128, 6, 4096]: plane j row p = feature j*128+p
        return np.ascontiguousarray(
            a.T.reshape(JT, 128, N).transpose(1, 0, 2))

    def chunks(a):
        # [128, 6, 4096] -> [1024, 3072]: chunk-major moving layout
        return np.ascontiguousarray(
            a.reshape(128, JT, NT, 512).transpose(2, 0, 1, 3)
            .reshape(NT * 128, JT * 512))

    m16 = chunks(planes(hi))                          # fp16
    mlo = chunks(planes(lo.astype(e5).astype(np.float32))).astype(e5)
    mhi = chunks(planes(flat.astype(e4).astype(np.float32))).astype(e4)
    nsq = np.stack([nsq_h, nsq_l]).astype(np.float16)

    st16_full = planes(hi2)                           # [128, 6, 4096] fp16
    sta_full = planes((2.0 * flat).astype(e4).astype(np.float32))
    stb_full = planes(lo2.astype(e5).astype(np.float32))

    def st_core(a, c, dt):
        return np.ascontiguousarray(
            a[:, :, c * M:(c + 1) * M].reshape(128, JT * 512)).astype(dt)

    return m16, mlo, mhi, nsq, st16_full, sta_full, stb_full, st_core


def kernel(x, k):
    assert int(k) == 16
    import ml_dtypes
    e4 = ml_dtypes.float8_e4m3
    e5 = ml_dtypes.float8_e5m2
    (m16, mlo, mhi, nsq,
     st16_full, sta_full, stb_full, st_core) = _prep_inputs(x)

    if "nc" not in _cache:
        _cache["nc"] = _build()
    nc = _cache["nc"]

    in_maps = [
        {"m16": m16, "mlo": mlo, "mhi": mhi, "nsq": nsq,
         "st16": st_core(st16_full, c, np.float16),
         "sta": st_core(sta_full, c, e4),
         "stb": st_core(stb_full, c, e5)}
        for c in range(NCORES)
    ]

    from concourse.bass_utils import run_bass_kernel_spmd
    trace = bool(os.environ.get("KNN_TRACE"))
    if trace:
        try:
            from antenv.axon_hooks import get_axon_ntff_profile_hook
            if get_axon_ntff_profile_hook() is None:
                trace = False
        except ImportError:
            trace = False
    res = run_bass_kernel_spmd(nc, in_maps, core_ids=list(range(NCORES)),
                               trace=trace)
    if trace and res.exec_time_ns is not None:
        print(f"HW exec time: {res.exec_time_ns} ns")
        _cache["exec_time_ns"] = res.exec_time_ns

    out = np.concatenate([r["out"] for r in res.results], axis=0)
    return out.reshape(B, S, S)


# revision 13
# speedup vs baseline: 1.6411x; 1.0886x over previous
"""kNN hypergraph kernel for Trainium2 (8 NeuronCores, Bass/Tile).

Problem: x [16, 256, 768] f32, k=16.
  flat = x.reshape(4096, 768)
  d2[i,j] = |flat_i - flat_j|^2 ; idx = 16 nearest (incl self)
  hypergraph[i, idx[i,:]] = 1 ; out[b,s,t] = sum_b2 hg[b*256+s, b2*256+t]
Output: [16, 256, 256] f32.

Strategy (row-sharded across 8 cores, 512 rows each):
  - Rank rows by s[i,j] = 2<x_i,x_j> - |x_j|^2 (per-row constant sq_i
    dropped). The 16 NN are the 16 LARGEST s per row.
  - s = hi2 @ hi'        (fp16 x fp16, full PE rate, 6 MMs/block)
      + e4m3(2x) @ e5m2(lo')   } both cross terms as fp8 DoubleRow
      + e5m2(lo2) @ e4m3(x')   } matmuls: K=256/instr, ~1.44x rate
      - sq  (fp16 hi/lo pair riding as one K=2 matmul)
    where hi2 = fp16(2x), lo2 = 2x - hi2, hi = fp16(x), lo = x - hi.
    Residual error ~1e-3 (vs top-16 decision gaps ~1e-1): exact ranking.
  - Column-chunk-outer loop (8 chunks of 512 cols x 4 row-tiles):
    each moving chunk is one contiguous-per-partition DMA, consumed by
    4 row-tiles -> DMA stays far ahead of PE.
  - Top-16 per row: DVE max8 per 512-block straight from PSUM (top-8 of
    each block; >8 of the true top-16 in one block is a ~1e-3 event and
    within the rel-err budget), then a tiny combine over the 8x8 union
    gives sigma = 16th largest. ACT drains PSUM->SBUF in parallel.
  - Mask (s >= sigma) + log-tree fold over the 16 batches of 256 cols;
    finalize work split across DVE and GpSimd to shorten the tail.
"""

import os

import numpy as np

B, S, D = 16, 256, 768
N = B * S            # 4096 points
NCORES = 8
M = N // NCORES      # 512 rows per core
JT = 6               # K planes of 128 (768 features)
NT = 8               # moving chunks of 512 columns
RT = M // 128        # 4 row-tiles of 128 per core
NEG = -3.0e38        # sentinel: far below any real s value

_cache = {}


def _build():
    import concourse.mybir as mybir
    import concourse.tile as tile
    from concourse import bacc

    f32 = mybir.dt.float32
    f16 = mybir.dt.float16
    bf16 = mybir.dt.bfloat16
    f8e4 = mybir.dt.float8e4
    f8e5 = mybir.dt.float8e5
    DR = mybir.MatmulPerfMode.DoubleRow

    nc = bacc.Bacc("TRN2", target_bir_lowering=False, debug=False,
                   num_devices=NCORES)

    # moving side (identical on all cores), chunk-major so each chunk is
    # one contiguous [128, 3072] DMA
    m16_d = nc.dram_tensor("m16", [NT * 128, JT * 512], f16,
                           kind="ExternalInput")
    mlo_d = nc.dram_tensor("mlo", [NT * 128, JT * 512], f8e5,
                           kind="ExternalInput")
    mhi_d = nc.dram_tensor("mhi", [NT * 128, JT * 512], f8e4,
                           kind="ExternalInput")
    # -sq pre-broadcast to 128 partitions (fp32 exact): the drain adds it
    nsq_d = nc.dram_tensor("nsq", [128, N], f32, kind="ExternalInput")
    # stationary side (this core's 512 rows)
    st16_d = nc.dram_tensor("st16", [128, JT * 512], f16,
                            kind="ExternalInput")
    sta_d = nc.dram_tensor("sta", [128, JT * 512], f8e4,
                           kind="ExternalInput")
    stb_d = nc.dram_tensor("stb", [128, JT * 512], f8e5,
                           kind="ExternalInput")
    out_d = nc.dram_tensor("out", [M, S], f32, kind="ExternalOutput")

    with tile.TileContext(nc) as tc:
        with (
            tc.tile_pool(name="weights", bufs=1) as wpool,
            tc.tile_pool(name="s", bufs=1) as spool,
            tc.tile_pool(name="m8", bufs=1) as m8pool,
            tc.tile_pool(name="fin", bufs=2) as fpool,
            tc.tile_pool(name="mask", bufs=1) as mpool,
            tc.tile_pool(name="outp", bufs=2) as opool,
            tc.tile_pool(name="psum", bufs=6, space="PSUM") as psum,
        ):
            # critical path first: the fp16 stationary + first moving chunk,
            # both split so round-0 matmuls can start before the full tile
            # lands (plane j is consumed in emission order)
            st16 = wpool.tile([128, JT, 512], f16, tag="st16", name="st16")
            nc.sync.dma_start(out=st16[:, 0:3, :], in_=st16_d[:, :3 * 512])
            m16_sb = [wpool.tile([128, JT, 512], f16, tag=f"m16_{n}",
                                 name=f"m16_{n}") for n in range(NT)]
            for h in range(3):
                nc.sync.dma_start(
                    out=m16_sb[0][:, 2 * h:2 * h + 2, :],
                    in_=m16_d[0:128, 2 * h * 512:(2 * h + 2) * 512])
            nc.sync.dma_start(out=st16[:, 3:6, :], in_=st16_d[:, 3 * 512:])
            sta = wpool.tile([128, JT, 512], f8e4, tag="sta", name="sta")
            nc.sync.dma_start(out=sta, in_=sta_d[:, :])
            stb = wpool.tile([128, JT, 512], f8e5, tag="stb", name="stb")
            nc.sync.dma_start(out=stb, in_=stb_d[:, :])
            mlo_sb = [wpool.tile([128, JT, 512], f8e5, tag=f"mlo_{n}",
                                 name=f"mlo_{n}") for n in range(NT)]
            mhi_sb = [wpool.tile([128, JT, 512], f8e4, tag=f"mhi_{n}",
                                 name=f"mhi_{n}") for n in range(NT)]
            nc.sync.dma_start(out=mlo_sb[0], in_=mlo_d[0:128, :])
            nc.sync.dma_start(out=mhi_sb[0], in_=mhi_d[0:128, :])
            nsq_sb = [wpool.tile([128, 512], f32, tag=f"nsq_{n}",
                                 name=f"nsq_{n}") for n in range(NT)]
            nc.sync.dma_start(out=nsq_sb[0], in_=nsq_d[:, 0:512])
            for n in range(1, NT):
                nc.sync.dma_start(out=m16_sb[n],
                                  in_=m16_d[n * 128:(n + 1) * 128, :])
                nc.sync.dma_start(out=mlo_sb[n],
                                  in_=mlo_d[n * 128:(n + 1) * 128, :])
                nc.sync.dma_start(out=mhi_sb[n],
                                  in_=mhi_d[n * 128:(n + 1) * 128, :])
                nc.sync.dma_start(out=nsq_sb[n],
                                  in_=nsq_d[:, n * 512:(n + 1) * 512])

            s_sb = [spool.tile([128, N], f32, tag=f"s{rt}", name=f"s{rt}")
                    for rt in range(RT)]
            m8s = [m8pool.tile([128, 64], f32, tag=f"m8_{rt}",
                               name=f"m8_{rt}") for rt in range(RT)]

            def finalize(rt):
                # sigma = 16th largest of the union of the 8 block top-8s
                c8 = fpool.tile([128, 8], f32, tag="c8", name="c8")
                m8x = fpool.tile([128, 64], f32, tag="m8x", name="m8x")
                d8 = fpool.tile([128, 8], f32, tag="d8", name="d8")
                nc.vector.max(out=c8, in_=m8s[rt])
                nc.vector.match_replace(out=m8x, in_to_replace=c8,
                                        in_values=m8s[rt], imm_value=NEG)
                nc.vector.max(out=d8, in_=m8x)
                sigma = d8[:, 7:8]

                # neighbor mask (s >= sigma): two single-ALU-pass is_ge
                # halves + one bf16 add beat one fused two-op pass
                eng = nc.vector
                H = N // 2
                mask = mpool.tile([128, H], bf16, tag="maskA",
                                  name=f"maskA{rt}")
                maskB = mpool.tile([128, H], bf16, tag="maskB",
                                   name=f"maskB{rt}")
                eng.tensor_scalar(mask, s_sb[rt][:, :H], sigma, None,
                                  op0=mybir.AluOpType.is_ge)
                eng.tensor_scalar(maskB, s_sb[rt][:, H:], sigma, None,
                                  op0=mybir.AluOpType.is_ge)
                eng.tensor_add(mask, mask, maskB)
                w = H // 2
                while w > S:
                    eng.tensor_add(mask[:, :w], mask[:, :w],
                                   mask[:, w:2 * w])
                    w //= 2
                o = opool.tile([128, S], f32, tag=f"o{rt % 2}",
                               name=f"o{rt}")
                eng.tensor_add(o, mask[:, :S], mask[:, S:2 * S])
                nc.sync.dma_start(
                    out=out_d[rt * 128:(rt + 1) * 128, :], in_=o)

            def block(n, rt):
                nsl = slice(n * 512, (n + 1) * 512)
                rsl = slice(rt * 128, (rt + 1) * 128)
                ps = psum.tile([128, 512], f32, tag="ps", name="ps")
                for j in range(JT):
                    nc.tensor.matmul(
                        ps, st16[:, j:j + 1, rsl],
                        m16_sb[n][:, j:j + 1, :],
                        start=(j == 0), stop=False)
                for c in range(JT // 2):
                    nc.tensor.matmul(
                        ps, sta[:, 2 * c:2 * c + 2, rsl],
                        mlo_sb[n][:, 2 * c:2 * c + 2, :],
                        start=False, stop=False, perf_mode=DR)
                for c in range(JT // 2):
                    nc.tensor.matmul(
                        ps, stb[:, 2 * c:2 * c + 2, rsl],
                        mhi_sb[n][:, 2 * c:2 * c + 2, :],
                        start=False, stop=(c == JT // 2 - 1), perf_mode=DR)
                # drain adds the exact fp32 -sq row; block top-8 follows
                nc.vector.tensor_add(s_sb[rt][:, nsl], ps, nsq_sb[n])
                nc.vector.max(out=m8s[rt][:, n * 8:(n + 1) * 8],
                              in_=s_sb[rt][:, nsl])

            # phase A: chunk-outer over the first NT/2 chunks (each moving
            # chunk is reused by all 4 row-tiles right after it lands)
            for n in range(NT // 2):
                for rt in range(RT):
                    block(n, rt)
            # phase B: row-outer so each row-tile finishes all its chunks
            # early and its finalize overlaps the next row-tile's matmuls
            for rt in range(RT):
                for n in range(NT // 2, NT):
                    block(n, rt)
                finalize(rt)

    nc.compile()
    return nc


def _prep_inputs(x):
    import ml_dtypes
    e4 = ml_dtypes.float8_e4m3
    e5 = ml_dtypes.float8_e5m2

    flat = np.asarray(x, dtype=np.float32).reshape(N, D)
    sq = (flat.astype(np.float64) ** 2).sum(1)

    hi = flat.astype(np.float16)                      # rhs fp16
    lo = flat - hi.astype(np.float32)                 # rhs residual
    hi2 = (2.0 * flat).astype(np.float16)             # lhs fp16
    lo2 = 2.0 * flat - hi2.astype(np.float32)         # lhs residual

    def planes(a):
        # [4096, 768] -> [128, 6, 4096]: plane j row p = feature j*128+p
        return np.ascontiguousarray(
            a.T.reshape(JT, 128, N).transpose(1, 0, 2))

    def chunks(a):
        # [128, 6, 4096] -> [1024, 3072]: chunk-major moving layout
        return np.ascontiguousarray(
            a.reshape(128, JT, NT, 512).transpose(2, 0, 1, 3)
            .reshape(NT * 128, JT * 512))

    m16 = chunks(planes(hi))                          # fp16
    mlo = chunks(planes(lo.astype(e5).astype(np.float32))).astype(e5)
    mhi = chunks(planes(flat.astype(e4).astype(np.float32))).astype(e4)
    nsq = np.ascontiguousarray(
        np.broadcast_to((-sq).astype(np.float32), (128, N)))

    st16_full = planes(hi2)                           # [128, 6, 4096] fp16
    sta_full = planes((2.0 * flat).astype(e4).astype(np.float32))
    stb_full = planes(lo2.astype(e5).astype(np.float32))

    def st_core(a, c, dt):
        return np.ascontiguousarray(
            a[:, :, c * M:(c + 1) * M].reshape(128, JT * 512)).astype(dt)

    return m16, mlo, mhi, nsq, st16_full, sta_full, stb_full, st_core


def kernel(x, k):
    assert int(k) == 16
    import ml_dtypes
    e4 = ml_dtypes.float8_e4m3
    e5 = ml_dtypes.float8_e5m2
    (m16, mlo, mhi, nsq,
     st16_full, sta_full, stb_full, st_core) = _prep_inputs(x)

    if "nc" not in _cache:
        _cache["nc"] = _build()
    nc = _cache["nc"]

    in_maps = [
        {"m16": m16, "mlo": mlo, "mhi": mhi, "nsq": nsq,
         "st16": st_core(st16_full, c, np.float16),
         "sta": st_core(sta_full, c, e4),
         "stb": st_core(stb_full, c, e5)}
        for c in range(NCORES)
    ]

    from concourse.bass_utils import run_bass_kernel_spmd
    trace = bool(os.environ.get("KNN_TRACE"))
    if trace:
        try:
            from antenv.axon_hooks import get_axon_ntff_profile_hook
            if get_axon_ntff_profile_hook() is None:
                trace = False
        except ImportError:
            trace = False
    res = run_bass_kernel_spmd(nc, in_maps, core_ids=list(range(NCORES)),
                               trace=trace)
    if trace and res.exec_time_ns is not None:
        print(f"HW exec time: {res.exec_time_ns} ns")
        _cache["exec_time_ns"] = res.exec_time_ns

    out = np.concatenate([r["out"] for r in res.results], axis=0)
    return out.reshape(B, S, S)
